# revision 27
# baseline (speedup 1.0000x reference)
"""Trainium2 Bass kernel for n-iteration Jacobi (3x3 cross stencil, reflect pad).

x_{t+1} = 0.25*(V + H) x_t + f,  f = COF*layout (|f| ~ 2.4e-9, contributes
< 3e-6 relative to the output; dropped).

V (vertical) and H (horizontal) neighbor-sum operators with this reflect
boundary are exactly diagonalized by the DCT-I basis v_k[i] = cos(pi*i*k/1023),
eigenvalues lam_k = 2*cos(pi*k/1023).  n Jacobi iterations collapse to one
spectral sandwich per image:

    out = C_k @ (Lam2D * (Cinv_k @ X @ Cinv_k^T)) @ C_k^T
    Lam2D[a,b] = ((lam_a + lam_b)/4)^n

Three reductions on top of the plain sandwich:
  1. Mode truncation: Lam2D^n decays doubly-exponentially away from the
     lowest/highest frequencies; keep K=512 of 1024 modes per axis for n=50
     (max truncated |Lam| ~ 4e-4).
  2. Even/odd folding: cos(pi*k*(1023-i)/1023) = (-1)^k cos(pi*k*i/1023),
     so folding the spatial axes into symmetric/antisymmetric halves halves
     every contraction.  The input fold is done on the host (images are sent
     as 4 parity quadrants), the intermediate parity recombines fuse into the
     PSUM copy-outs as add/sub pairs, and the output unfold is a host-side
     index permutation.
  3. No PE transposes: the two passes that would need transposed outputs
     (forward-vertical, inverse-vertical) run with the *data* as the
     stationary lhsT operand and the transform matrix as the moving rhs,
     which yields the transposed orientation directly.

Per image: 4 half-contraction matmul passes, ~49K PE rows at 1 row/cycle
(vs ~1.25M rows for iterated banded-matmul stepping).  All matmul operands
fp16 (PSUM accumulates fp32); measured error vs the fp64 reference ~7e-4
max-rel.  Per core: 2 of 16 images, passes software-pipelined across the
two images.
"""

import math
from contextlib import ExitStack

import numpy as np

NX = 1024
N_CORES = 8
IMGS_PER_CORE = 2
LN_TAU = math.log(1e4)

_compiled_cache = {}


def _choose_K(n_iter):
    # keep modes with ((lam_a+lam_b)/4)^n >= 1e-4; parity folding needs
    # K to be a multiple of 256
    R = int(math.ceil(1023.0 / math.pi * math.sqrt(2.0 * LN_TAU / max(n_iter, 1))))
    K = min(1024, ((2 * R + 255) // 256) * 256)
    return K


def _host_weights(n_iter, K):
    i = np.arange(NX)
    C = np.cos(np.pi * np.outer(i, i) / (NX - 1))
    lam = 2.0 * np.cos(np.pi * i / (NX - 1))
    w = np.ones(NX)
    w[0] = w[-1] = 0.5
    s = math.sqrt(2.0 / (NX - 1))
    # C^{-1} = (2/(N-1)) W C W; balance fp16 range: A1 = Cinv/s, B1 = C*s
    A1 = (2.0 / (NX - 1) / s) * (w[:, None] * C * w[None, :])
    B1 = C * s
    R = K // 2
    kept = np.r_[0:R, NX - R:NX]
    kperm = np.r_[kept[kept % 2 == 0], kept[kept % 2 == 1]]  # evens, then odds
    A1t = A1[kperm, :512].T               # [512 (i'/j'), K]   fwd weights
    B1t = B1[:512, :][:, kperm].T         # [K, 512 (j'/i')]   inv weights
    Lam = ((lam[kperm][:, None] + lam[kperm][None, :]) / 4.0) ** n_iter
    KB = K // 128
    # WA[c, cblk*K + k]   = A1t[cblk*128 + c, k]      (cblk: spatial block)
    # WB[c, kblk*512 + f] = B1t[kblk*128 + c, f]      (kblk: mode block)
    WA = A1t.reshape(4, 128, K).transpose(1, 0, 2)
    WA = np.ascontiguousarray(WA.reshape(128, 4 * K)).astype(np.float16)
    WB = B1t.reshape(KB, 128, 512).transpose(1, 0, 2)
    WB = np.ascontiguousarray(WB.reshape(128, KB * 512)).astype(np.float16)
    LAM = Lam.reshape(KB, 128, K).transpose(1, 0, 2)
    LAM = np.ascontiguousarray(LAM.reshape(128, KB * K)).astype(np.float32)
    return {"wa": WA, "wb": WB, "lam": LAM}


def _build_program(n_iter):
    import concourse.bacc as bacc
    import concourse.mybir as mybir
    import concourse.tile as tile

    K = _choose_K(n_iter)
    KB = K // 128          # mode blocks (parity-permuted: KB/2 even, KB/2 odd)
    KH = KB // 2           # blocks per parity
    KP = K // 2            # modes per parity
    nslots = 2 if K <= 512 else 1
    f16 = mybir.dt.float16
    f32 = mybir.dt.float32
    mult = mybir.AluOpType.mult
    add = mybir.AluOpType.add
    sub = mybir.AluOpType.subtract

    nc = bacc.Bacc("TRN2", target_bir_lowering=False, debug=False)
    # x0: per image the exact SBUF layout [128, 16*512] (quadrant q, block ci
    # at cols (q*4+ci)*512); shape-preserving DMAs only
    x0_d = nc.dram_tensor("x0", [IMGS_PER_CORE * 128, 16 * 512], f16,
                          kind="ExternalInput").ap()
    wa_d = nc.dram_tensor("wa", [128, 4 * K], f16, kind="ExternalInput").ap()
    wb_d = nc.dram_tensor("wb", [128, KB * 512], f16,
                          kind="ExternalInput").ap()
    lam_d = nc.dram_tensor("lam", [128, KB * K], f32, kind="ExternalInput").ap()
    # y: raw folded output [1024, 1024] per image (host unfolds)
    y_d = nc.dram_tensor("y", [IMGS_PER_CORE * NX, NX], f16,
                         kind="ExternalOutput").ap()

    with tile.TileContext(nc) as tc, ExitStack() as ctx:
        wp = ctx.enter_context(tc.tile_pool(name="w", bufs=1))
        bp = ctx.enter_context(tc.tile_pool(name="b", bufs=1))
        psum_bufs = 8 if K <= 512 else 4
        pmm = ctx.enter_context(tc.tile_pool(name="pmm", bufs=psum_bufs,
                                             space="PSUM"))
        sp = ctx.enter_context(tc.tile_pool(name="sp", bufs=4))

        WA = wp.tile([128, 4 * K], f16)
        WB = wp.tile([128, KB * 512], f16)
        LAM = wp.tile([128, KB * K], f32)

        # Xq: 16 blocks of [128, 512]: quadrant q (a=i-parity, b=j-parity,
        # q = 2a + b), block = q*4 + ci
        Xq = [bp.tile([128, 16 * 512], f16, name=f"x{s}") for s in range(nslots)]
        # UT_p[j', kv] (vertical modes already transposed): col = jb*K + kv
        UTe = [bp.tile([128, 4 * K], f16, name=f"ute{s}") for s in range(nslots)]
        UTo = [bp.tile([128, 4 * K], f16, name=f"uto{s}") for s in range(nslots)]
        # WC[kh, kv] scaled by Lam: col = khblk*K + kv
        WC = [bp.tile([128, KB * K], f16, name=f"wc{s}") for s in range(nslots)]
        # ZT[kv, col]: col<512 = j' (sym part), col>=512 = j' (antisym part)
        ZT = [bp.tile([128, KB * NX], f16, name=f"zt{s}") for s in range(nslots)]
        Ol = [bp.tile([128, 4 * NX], f16, name=f"ol{s}") for s in range(nslots)]
        Oh = [bp.tile([128, 4 * NX], f16, name=f"oh{s}") for s in range(nslots)]

        def load_x(s, img):
            r0 = img * 128
            for h in range(2):           # one DMA per pj half [128, 4096]
                nc.sync.dma_start(Xq[s][:, h * 4096:(h + 1) * 4096],
                                  x0_d[r0:r0 + 128, h * 4096:(h + 1) * 4096])

        def passA(s):
            # UT_pj[j', k] = sum_{i'} Xq[par(k), pj][i', j'] * A1t[i', k]
            # lhsT = input quadrant block, rhs = WA slice; even/odd k halves
            # accumulate into the two col-halves of one PSUM bank
            for pj in range(2):
                for jb in range(4):
                    p = pmm.tile([128, 2 * KP], f32, name="pm", tag="mm")
                    for par in range(2):           # k parity: even, odd
                        pos = 2 * pj + par
                        for ci in range(4):
                            nc.tensor.matmul(
                                p[:, par * KP:(par + 1) * KP],
                                Xq[s][:, (pos * 4 + ci) * 512 + jb * 128:
                                       (pos * 4 + ci) * 512 + jb * 128 + 128],
                                WA[:, ci * K + par * KP: ci * K + (par + 1) * KP],
                                start=(ci == 0), stop=(ci == 3))
                    dst = UTe[s] if pj == 0 else UTo[s]
                    nc.scalar.copy(dst[:, jb * K:(jb + 1) * K], p[:])

        def passC(s):
            # WC[kh, kv] = Lam * sum_{j'} A1p[kh, j'] UT_{par(kh)}[j', kv]
            for ko in range(KB):
                rhs = UTe[s] if ko < KH else UTo[s]
                for f0 in range(0, K, 512):
                    fw = min(512, K - f0)
                    p = pmm.tile([128, fw], f32, name="pm", tag="mm")
                    for jb in range(4):
                        nc.tensor.matmul(
                            p[:], WA[:, jb * K + ko * 128: jb * K + ko * 128 + 128],
                            rhs[:, jb * K + f0: jb * K + f0 + fw],
                            start=(jb == 0), stop=(jb == 3))
                    nc.vector.tensor_tensor(
                        WC[s][:, ko * K + f0: ko * K + f0 + fw], p[:],
                        LAM[:, ko * K + f0: ko * K + f0 + fw], op=mult)

        def passE(s):
            # ZeT/ZoT[kv, j'] = sum_{kh even/odd} WC[kh, kv] B1t[kh, j']
            # lhsT = WC block, rhs = WB slice.
            # ZT[:, kb*NX + :512] = ZeT + ZoT (Z at j'), + 512: = ZeT - ZoT
            for kvb in range(KB):
                pe = pmm.tile([128, 512], f32, name="pe", tag="mm")
                po = pmm.tile([128, 512], f32, name="po", tag="mm")
                for kb in range(KH):
                    nc.tensor.matmul(
                        pe[:], WC[s][:, kb * K + kvb * 128: kb * K + kvb * 128 + 128],
                        WB[:, kb * 512:(kb + 1) * 512],
                        start=(kb == 0), stop=(kb == KH - 1))
                for kb in range(KH, KB):
                    nc.tensor.matmul(
                        po[:], WC[s][:, kb * K + kvb * 128: kb * K + kvb * 128 + 128],
                        WB[:, kb * 512:(kb + 1) * 512],
                        start=(kb == KH), stop=(kb == KB - 1))
                ps = sp.tile([128, 512], f16, name="ps", tag="ps")
                nc.scalar.mul(ps[:], po[:], 2.0)
                nc.vector.scalar_tensor_tensor(
                    ZT[s][:, kvb * NX: kvb * NX + 512], ps[:], 0.5, pe[:],
                    op0=mult, op1=add)
                nc.gpsimd.tensor_tensor(
                    ZT[s][:, kvb * NX + 512: (kvb + 1) * NX],
                    ZT[s][:, kvb * NX: kvb * NX + 512], ps[:], op=sub)

        def passG(s, img):
            # Oe/Oo[i', col] = sum_{kv even/odd} B1p[i', kv] ZT[kv, col]
            # Ol = Oe + Oo (= out[i']), Oh = Oe - Oo (= out[1023-i'])
            r0 = img * NX
            for io in range(4):
                for hf in range(2):
                    pe = pmm.tile([128, 512], f32, name="pe", tag="mm")
                    po = pmm.tile([128, 512], f32, name="po", tag="mm")
                    for kb in range(KH):
                        o = kb * 512 + io * 128
                        nc.tensor.matmul(
                            pe[:], WB[:, o:o + 128],
                            ZT[s][:, kb * NX + hf * 512: kb * NX + hf * 512 + 512],
                            start=(kb == 0), stop=(kb == KH - 1))
                    for kb in range(KH, KB):
                        o = kb * 512 + io * 128
                        nc.tensor.matmul(
                            po[:], WB[:, o:o + 128],
                            ZT[s][:, kb * NX + hf * 512: kb * NX + hf * 512 + 512],
                            start=(kb == KH), stop=(kb == KB - 1))
                    c0 = io * NX + hf * 512
                    ps = sp.tile([128, 512], f16, name="ps", tag="ps")
                    nc.scalar.mul(ps[:], po[:], 2.0)
                    nc.vector.scalar_tensor_tensor(
                        Ol[s][:, c0:c0 + 512], ps[:], 0.5, pe[:],
                        op0=mult, op1=add)
                    nc.gpsimd.tensor_tensor(
                        Oh[s][:, c0:c0 + 512], Ol[s][:, c0:c0 + 512], ps[:],
                        op=sub)
                nc.sync.dma_start(y_d[r0 + io * 128: r0 + (io + 1) * 128, :],
                                  Ol[s][:, io * NX:(io + 1) * NX])
                nc.scalar.dma_start(
                    y_d[r0 + 512 + io * 128: r0 + 512 + (io + 1) * 128, :],
                    Oh[s][:, io * NX:(io + 1) * NX])

        # weights on the ACT HWDGE queue so issue/transfer overlaps X loads
        nc.scalar.dma_start(WA[:], wa_d[:, :])
        load_x(0, 0)
        nc.scalar.dma_start(WB[:], wb_d[:, :])
        nc.scalar.dma_start(LAM[:], lam_d[:, :])
        if nslots == 2:
            load_x(1, 1)
            passA(0); passA(1)
            passC(0); passC(1)
            passE(0); passE(1)
            passG(0, 0); passG(1, 1)
        else:
            for img in range(IMGS_PER_CORE):
                if img:
                    load_x(0, img)
                passA(0); passC(0); passE(0); passG(0, img)

    nc.compile()
    return nc, _host_weights(n_iter, _choose_K(n_iter))


def _fold_input(x_f32):
    """[16, NX, NX] f32 -> [16, 128, 8192] f16 parity quadrants in the
    device SBUF layout: col = (q*4 + ci)*512 + j', partition = i' % 128."""
    lo = x_f32[:, :512, :]
    hi = x_f32[:, 1023:511:-1, :]
    ia = lo + hi    # i-even
    ib = lo - hi    # i-odd
    quad = np.empty((16, 4, 512, 512), np.float32)
    for q, part in ((0, ia), (2, ib)):
        quad[:, q] = part[:, :, :512] + part[:, :, 1023:511:-1]
        quad[:, q + 1] = part[:, :, :512] - part[:, :, 1023:511:-1]
    quad = quad[:, [0, 2, 1, 3]]     # pj-major device order
    # [16, qpos, ci*128+p, j'] -> [16, p, qpos, ci, j']
    quad = quad.reshape(16, 4, 4, 128, 512).transpose(0, 3, 1, 2, 4)
    return np.ascontiguousarray(quad.reshape(16, 128, 8192)).astype(np.float16)


_PERM = np.r_[0:512, 1023:511:-1]


def _make_in_maps(x_f32, n_iter):
    """x_f32: [16, NX, NX] float32. Returns (nc, in_maps)."""
    if n_iter not in _compiled_cache:
        _compiled_cache[n_iter] = _build_program(n_iter)
    nc, wdict = _compiled_cache[n_iter]
    xq = _fold_input(x_f32)
    in_maps = []
    for c in range(N_CORES):
        shard = np.ascontiguousarray(
            xq[c * IMGS_PER_CORE:(c + 1) * IMGS_PER_CORE].reshape(
                IMGS_PER_CORE * 128, 16 * 512))
        m = {"x0": shard}
        m.update(wdict)
        in_maps.append(m)
    return nc, in_maps


def kernel(layout, heat, n_iter):
    n_iter = int(n_iter)
    heat = np.asarray(heat)
    out_shape = heat.shape
    x = np.asarray(heat, np.float32).reshape(16, NX, NX)
    if n_iter <= 0:
        return x.reshape(out_shape).copy()

    from concourse.bass_utils import run_bass_kernel_spmd

    nc, in_maps = _make_in_maps(x, n_iter)
    res = run_bass_kernel_spmd(nc, in_maps, core_ids=list(range(N_CORES)))
    out = np.empty((16, NX, NX), np.float32)
    for c in range(N_CORES):
        raw = res.results[c]["y"].astype(np.float32).reshape(
            IMGS_PER_CORE, NX, NX)
        # unfold: raw row/col r>=512 holds index 1535-r
        out[c * IMGS_PER_CORE:(c + 1) * IMGS_PER_CORE] = (
            raw[:, _PERM][:, :, _PERM])
    return out.reshape(out_shape)


# revision 28
# speedup vs baseline: 1.0754x; 1.0754x over previous
"""Trainium2 Bass kernel for n-iteration Jacobi (3x3 cross stencil, reflect pad).

x_{t+1} = 0.25*(V + H) x_t + f,  f = COF*layout (|f| ~ 2.4e-9, contributes
< 3e-6 relative to the output; dropped).

V (vertical) and H (horizontal) neighbor-sum operators with this reflect
boundary are exactly diagonalized by the DCT-I basis v_k[i] = cos(pi*i*k/1023),
eigenvalues lam_k = 2*cos(pi*k/1023).  n Jacobi iterations collapse to one
spectral sandwich per image:

    out = C_k @ (Lam2D * (Cinv_k @ X @ Cinv_k^T)) @ C_k^T
    Lam2D[a,b] = ((lam_a + lam_b)/4)^n

Three reductions on top of the plain sandwich:
  1. Mode truncation: Lam2D^n decays doubly-exponentially away from the
     lowest/highest frequencies; keep K=512 of 1024 modes per axis for n=50
     (max truncated |Lam| ~ 4e-4).
  2. Even/odd folding: cos(pi*k*(1023-i)/1023) = (-1)^k cos(pi*k*i/1023),
     so folding the spatial axes into symmetric/antisymmetric halves halves
     every contraction.  The input fold is done on the host (images are sent
     as 4 parity quadrants), the intermediate parity recombines fuse into the
     PSUM copy-outs as add/sub pairs, and the output unfold is a host-side
     index permutation.
  3. No PE transposes: the two passes that would need transposed outputs
     (forward-vertical, inverse-vertical) run with the *data* as the
     stationary lhsT operand and the transform matrix as the moving rhs,
     which yields the transposed orientation directly.

Per image: 4 half-contraction matmul passes, ~49K PE rows at 1 row/cycle
(vs ~1.25M rows for iterated banded-matmul stepping).  All matmul operands
fp16 (PSUM accumulates fp32); measured error vs the fp64 reference ~7e-4
max-rel.  Per core: 2 of 16 images, passes software-pipelined across the
two images.
"""

import math
from contextlib import ExitStack

import numpy as np

NX = 1024
N_CORES = 8
IMGS_PER_CORE = 2
LN_TAU = math.log(1e4)

_compiled_cache = {}


def _choose_K(n_iter):
    # keep modes with ((lam_a+lam_b)/4)^n >= 1e-4; parity folding needs
    # K to be a multiple of 256
    R = int(math.ceil(1023.0 / math.pi * math.sqrt(2.0 * LN_TAU / max(n_iter, 1))))
    K = min(1024, ((2 * R + 255) // 256) * 256)
    return K


def _host_weights(n_iter, K):
    i = np.arange(NX)
    C = np.cos(np.pi * np.outer(i, i) / (NX - 1))
    lam = 2.0 * np.cos(np.pi * i / (NX - 1))
    w = np.ones(NX)
    w[0] = w[-1] = 0.5
    s = math.sqrt(2.0 / (NX - 1))
    # C^{-1} = (2/(N-1)) W C W; balance fp16 range: A1 = Cinv/s, B1 = C*s
    A1 = (2.0 / (NX - 1) / s) * (w[:, None] * C * w[None, :])
    B1 = C * s
    R = K // 2
    kept = np.r_[0:R, NX - R:NX]
    kperm = np.r_[kept[kept % 2 == 0], kept[kept % 2 == 1]]  # evens, then odds
    A1t = A1[kperm, :512].T               # [512 (i'/j'), K]   fwd weights
    B1t = B1[:512, :][:, kperm].T         # [K, 512 (j'/i')]   inv weights
    Lam = ((lam[kperm][:, None] + lam[kperm][None, :]) / 4.0) ** n_iter
    KB = K // 128
    # WA[c, cblk*K + k]   = A1t[cblk*128 + c, k]      (cblk: spatial block)
    # WB[c, kblk*512 + f] = B1t[kblk*128 + c, f]      (kblk: mode block)
    WA = A1t.reshape(4, 128, K).transpose(1, 0, 2)
    WA = np.ascontiguousarray(WA.reshape(128, 4 * K)).astype(np.float16)
    WB = B1t.reshape(KB, 128, 512).transpose(1, 0, 2)
    WB = np.ascontiguousarray(WB.reshape(128, KB * 512)).astype(np.float16)
    LAM = Lam.reshape(KB, 128, K).transpose(1, 0, 2)
    LAM = np.ascontiguousarray(LAM.reshape(128, KB * K)).astype(np.float32)
    return {"wa": WA, "wb": WB, "lam": LAM}


def _build_program(n_iter):
    import concourse.bacc as bacc
    import concourse.mybir as mybir
    import concourse.tile as tile

    K = _choose_K(n_iter)
    KB = K // 128          # mode blocks (parity-permuted: KB/2 even, KB/2 odd)
    KH = KB // 2           # blocks per parity
    KP = K // 2            # modes per parity
    nslots = 2 if K <= 512 else 1
    f16 = mybir.dt.float16
    f32 = mybir.dt.float32
    mult = mybir.AluOpType.mult
    add = mybir.AluOpType.add
    sub = mybir.AluOpType.subtract

    nc = bacc.Bacc("TRN2", target_bir_lowering=False, debug=False)
    # x0: per image the exact SBUF layout [128, 16*512] (quadrant q, block ci
    # at cols (q*4+ci)*512); shape-preserving DMAs only
    x0_d = nc.dram_tensor("x0", [IMGS_PER_CORE * 128, 16 * 512], f16,
                          kind="ExternalInput").ap()
    wa_d = nc.dram_tensor("wa", [128, 4 * K], f16, kind="ExternalInput").ap()
    wb_d = nc.dram_tensor("wb", [128, KB * 512], f16,
                          kind="ExternalInput").ap()
    lam_d = nc.dram_tensor("lam", [128, KB * K], f32, kind="ExternalInput").ap()
    # y: raw folded output [1024, 1024] per image (host unfolds)
    y_d = nc.dram_tensor("y", [IMGS_PER_CORE * NX, NX], f16,
                         kind="ExternalOutput").ap()

    with tile.TileContext(nc) as tc, ExitStack() as ctx:
        wp = ctx.enter_context(tc.tile_pool(name="w", bufs=1))
        bp = ctx.enter_context(tc.tile_pool(name="b", bufs=1))
        psum_bufs = 8 if K <= 512 else 4
        pmm = ctx.enter_context(tc.tile_pool(name="pmm", bufs=psum_bufs,
                                             space="PSUM"))
        sp = ctx.enter_context(tc.tile_pool(name="sp", bufs=6))

        WA = wp.tile([128, 4 * K], f16)
        WB = wp.tile([128, KB * 512], f16)
        LAM = wp.tile([128, KB * K], f32)

        # Xq: 16 blocks of [128, 512]: quadrant q (a=i-parity, b=j-parity,
        # q = 2a + b), block = q*4 + ci
        Xq = [bp.tile([128, 16 * 512], f16, name=f"x{s}") for s in range(nslots)]
        # UT_p[j', kv] (vertical modes already transposed): col = jb*K + kv
        UTe = [bp.tile([128, 4 * K], f16, name=f"ute{s}") for s in range(nslots)]
        UTo = [bp.tile([128, 4 * K], f16, name=f"uto{s}") for s in range(nslots)]
        # WC[kh, kv] scaled by Lam: col = khblk*K + kv
        WC = [bp.tile([128, KB * K], f16, name=f"wc{s}") for s in range(nslots)]
        # ZT[kv, col]: col<512 = j' (sym part), col>=512 = j' (antisym part)
        ZT = [bp.tile([128, KB * NX], f16, name=f"zt{s}") for s in range(nslots)]
        Ol = [bp.tile([128, 4 * NX], f16, name=f"ol{s}") for s in range(nslots)]
        Oh = [bp.tile([128, 4 * NX], f16, name=f"oh{s}") for s in range(nslots)]

        def load_x(s, img):
            r0 = img * 128
            for h in range(2):           # one DMA per pj half [128, 4096]
                nc.sync.dma_start(Xq[s][:, h * 4096:(h + 1) * 4096],
                                  x0_d[r0:r0 + 128, h * 4096:(h + 1) * 4096])

        def passA(s):
            # UT_pj[j', k] = sum_{i'} Xq[par(k), pj][i', j'] * A1t[i', k]
            # lhsT = input quadrant block, rhs = WA slice; even/odd k halves
            # accumulate into the two col-halves of one PSUM bank
            for pj in range(2):
                for jb in range(4):
                    p = pmm.tile([128, 2 * KP], f32, name="pm", tag="mm")
                    for par in range(2):           # k parity: even, odd
                        pos = 2 * pj + par
                        for ci in range(4):
                            nc.tensor.matmul(
                                p[:, par * KP:(par + 1) * KP],
                                Xq[s][:, (pos * 4 + ci) * 512 + jb * 128:
                                       (pos * 4 + ci) * 512 + jb * 128 + 128],
                                WA[:, ci * K + par * KP: ci * K + (par + 1) * KP],
                                start=(ci == 0), stop=(ci == 3))
                    dst = UTe[s] if pj == 0 else UTo[s]
                    nc.scalar.copy(dst[:, jb * K:(jb + 1) * K], p[:])

        def passC(s):
            # WC[kh, kv] = Lam * sum_{j'} A1p[kh, j'] UT_{par(kh)}[j', kv]
            for ko in range(KB):
                rhs = UTe[s] if ko < KH else UTo[s]
                for f0 in range(0, K, 512):
                    fw = min(512, K - f0)
                    p = pmm.tile([128, fw], f32, name="pm", tag="mm")
                    for jb in range(4):
                        nc.tensor.matmul(
                            p[:], WA[:, jb * K + ko * 128: jb * K + ko * 128 + 128],
                            rhs[:, jb * K + f0: jb * K + f0 + fw],
                            start=(jb == 0), stop=(jb == 3))
                    nc.vector.tensor_tensor(
                        WC[s][:, ko * K + f0: ko * K + f0 + fw], p[:],
                        LAM[:, ko * K + f0: ko * K + f0 + fw], op=mult)

        def passE(s):
            # ZeT/ZoT[kv, j'] = sum_{kh even/odd} WC[kh, kv] B1t[kh, j']
            # lhsT = WC block, rhs = WB slice.
            # ZT[:, kb*NX + :512] = ZeT + ZoT (Z at j'), + 512: = ZeT - ZoT
            for kvb in range(KB):
                pe = pmm.tile([128, 512], f32, name="pe", tag="mm")
                po = pmm.tile([128, 512], f32, name="po", tag="mm")
                for kb in range(KH):
                    nc.tensor.matmul(
                        pe[:], WC[s][:, kb * K + kvb * 128: kb * K + kvb * 128 + 128],
                        WB[:, kb * 512:(kb + 1) * 512],
                        start=(kb == 0), stop=(kb == KH - 1))
                for kb in range(KH, KB):
                    nc.tensor.matmul(
                        po[:], WC[s][:, kb * K + kvb * 128: kb * K + kvb * 128 + 128],
                        WB[:, kb * 512:(kb + 1) * 512],
                        start=(kb == KH), stop=(kb == KB - 1))
                ps = sp.tile([128, 512], f16, name="ps", tag="ps")
                nc.scalar.mul(ps[:], po[:], 2.0)
                nc.vector.scalar_tensor_tensor(
                    ZT[s][:, kvb * NX: kvb * NX + 512], ps[:], 0.5, pe[:],
                    op0=mult, op1=add)
                nc.gpsimd.tensor_tensor(
                    ZT[s][:, kvb * NX + 512: (kvb + 1) * NX],
                    ZT[s][:, kvb * NX: kvb * NX + 512], ps[:], op=sub)

        def passG(s, img):
            # Oe/Oo[i', col] = sum_{kv even/odd} B1p[i', kv] ZT[kv, col]
            # Ol = Oe + Oo (= out[i']), Oh = Oe - Oo (= out[1023-i'])
            r0 = img * NX
            for io in range(4):
                for hf in range(2):
                    pe = pmm.tile([128, 512], f32, name="pe", tag="mm")
                    po = pmm.tile([128, 512], f32, name="po", tag="mm")
                    for kb in range(KH):
                        o = kb * 512 + io * 128
                        nc.tensor.matmul(
                            pe[:], WB[:, o:o + 128],
                            ZT[s][:, kb * NX + hf * 512: kb * NX + hf * 512 + 512],
                            start=(kb == 0), stop=(kb == KH - 1))
                    for kb in range(KH, KB):
                        o = kb * 512 + io * 128
                        nc.tensor.matmul(
                            po[:], WB[:, o:o + 128],
                            ZT[s][:, kb * NX + hf * 512: kb * NX + hf * 512 + 512],
                            start=(kb == KH), stop=(kb == KB - 1))
                    c0 = io * NX + hf * 512
                    ps = sp.tile([128, 512], f16, name="ps", tag="ps")
                    nc.scalar.mul(ps[:], po[:], 2.0)
                    nc.vector.scalar_tensor_tensor(
                        Ol[s][:, c0:c0 + 512], ps[:], 0.5, pe[:],
                        op0=mult, op1=add)
                    nc.gpsimd.tensor_tensor(
                        Oh[s][:, c0:c0 + 512], Ol[s][:, c0:c0 + 512], ps[:],
                        op=sub)
                nc.sync.dma_start(y_d[r0 + io * 128: r0 + (io + 1) * 128, :],
                                  Ol[s][:, io * NX:(io + 1) * NX])
                nc.sync.dma_start(
                    y_d[r0 + 512 + io * 128: r0 + 512 + (io + 1) * 128, :],
                    Oh[s][:, io * NX:(io + 1) * NX])

        # weights on the ACT HWDGE queue so issue/transfer overlaps X loads
        nc.scalar.dma_start(WA[:], wa_d[:, :])
        load_x(0, 0)
        nc.scalar.dma_start(WB[:], wb_d[:, :])
        nc.scalar.dma_start(LAM[:], lam_d[:, :])
        if nslots == 2:
            load_x(1, 1)
            passA(0); passA(1)
            passC(0); passC(1)
            passE(0); passE(1)
            passG(0, 0); passG(1, 1)
        else:
            for img in range(IMGS_PER_CORE):
                if img:
                    load_x(0, img)
                passA(0); passC(0); passE(0); passG(0, img)

    nc.compile()
    return nc, _host_weights(n_iter, _choose_K(n_iter))


def _fold_input(x_f32):
    """[16, NX, NX] f32 -> [16, 128, 8192] f16 parity quadrants in the
    device SBUF layout: col = (q*4 + ci)*512 + j', partition = i' % 128."""
    lo = x_f32[:, :512, :]
    hi = x_f32[:, 1023:511:-1, :]
    ia = lo + hi    # i-even
    ib = lo - hi    # i-odd
    quad = np.empty((16, 4, 512, 512), np.float32)
    for q, part in ((0, ia), (2, ib)):
        quad[:, q] = part[:, :, :512] + part[:, :, 1023:511:-1]
        quad[:, q + 1] = part[:, :, :512] - part[:, :, 1023:511:-1]
    quad = quad[:, [0, 2, 1, 3]]     # pj-major device order
    # [16, qpos, ci*128+p, j'] -> [16, p, qpos, ci, j']
    quad = quad.reshape(16, 4, 4, 128, 512).transpose(0, 3, 1, 2, 4)
    return np.ascontiguousarray(quad.reshape(16, 128, 8192)).astype(np.float16)


_PERM = np.r_[0:512, 1023:511:-1]


def _make_in_maps(x_f32, n_iter):
    """x_f32: [16, NX, NX] float32. Returns (nc, in_maps)."""
    if n_iter not in _compiled_cache:
        _compiled_cache[n_iter] = _build_program(n_iter)
    nc, wdict = _compiled_cache[n_iter]
    xq = _fold_input(x_f32)
    in_maps = []
    for c in range(N_CORES):
        shard = np.ascontiguousarray(
            xq[c * IMGS_PER_CORE:(c + 1) * IMGS_PER_CORE].reshape(
                IMGS_PER_CORE * 128, 16 * 512))
        m = {"x0": shard}
        m.update(wdict)
        in_maps.append(m)
    return nc, in_maps


def kernel(layout, heat, n_iter):
    n_iter = int(n_iter)
    heat = np.asarray(heat)
    out_shape = heat.shape
    x = np.asarray(heat, np.float32).reshape(16, NX, NX)
    if n_iter <= 0:
        return x.reshape(out_shape).copy()

    from concourse.bass_utils import run_bass_kernel_spmd

    nc, in_maps = _make_in_maps(x, n_iter)
    res = run_bass_kernel_spmd(nc, in_maps, core_ids=list(range(N_CORES)))
    out = np.empty((16, NX, NX), np.float32)
    for c in range(N_CORES):
        raw = res.results[c]["y"].astype(np.float32).reshape(
            IMGS_PER_CORE, NX, NX)
        # unfold: raw row/col r>=512 holds index 1535-r
        out[c * IMGS_PER_CORE:(c + 1) * IMGS_PER_CORE] = (
            raw[:, _PERM][:, :, _PERM])
    return out.reshape(out_shape)


# revision 29
# speedup vs baseline: 1.0904x; 1.0140x over previous
"""Trainium2 Bass kernel for n-iteration Jacobi (3x3 cross stencil, reflect pad).

x_{t+1} = 0.25*(V + H) x_t + f,  f = COF*layout (|f| ~ 2.4e-9, contributes
< 3e-6 relative to the output; dropped).

V (vertical) and H (horizontal) neighbor-sum operators with this reflect
boundary are exactly diagonalized by the DCT-I basis v_k[i] = cos(pi*i*k/1023),
eigenvalues lam_k = 2*cos(pi*k/1023).  n Jacobi iterations collapse to one
spectral sandwich per image:

    out = C_k @ (Lam2D * (Cinv_k @ X @ Cinv_k^T)) @ C_k^T
    Lam2D[a,b] = ((lam_a + lam_b)/4)^n

Three reductions on top of the plain sandwich:
  1. Mode truncation: Lam2D^n decays doubly-exponentially away from the
     lowest/highest frequencies; keep K=512 of 1024 modes per axis for n=50
     (max truncated |Lam| ~ 4e-4).
  2. Even/odd folding: cos(pi*k*(1023-i)/1023) = (-1)^k cos(pi*k*i/1023),
     so folding the spatial axes into symmetric/antisymmetric halves halves
     every contraction.  The input fold is done on the host (images are sent
     as 4 parity quadrants), the intermediate parity recombines fuse into the
     PSUM copy-outs as add/sub pairs, and the output unfold is a host-side
     index permutation.
  3. No PE transposes: the two passes that would need transposed outputs
     (forward-vertical, inverse-vertical) run with the *data* as the
     stationary lhsT operand and the transform matrix as the moving rhs,
     which yields the transposed orientation directly.

Per image: 4 half-contraction matmul passes, ~49K PE rows at 1 row/cycle
(vs ~1.25M rows for iterated banded-matmul stepping).  All matmul operands
fp16 (PSUM accumulates fp32); measured error vs the fp64 reference ~7e-4
max-rel.  Per core: 2 of 16 images, passes software-pipelined across the
two images.
"""

import math
from contextlib import ExitStack

import numpy as np

NX = 1024
N_CORES = 8
IMGS_PER_CORE = 2
LN_TAU = math.log(1e4)

_compiled_cache = {}


def _choose_K(n_iter):
    # keep modes with ((lam_a+lam_b)/4)^n >= 1e-4; parity folding needs
    # K to be a multiple of 256
    R = int(math.ceil(1023.0 / math.pi * math.sqrt(2.0 * LN_TAU / max(n_iter, 1))))
    K = min(1024, ((2 * R + 255) // 256) * 256)
    return K


def _host_weights(n_iter, K):
    i = np.arange(NX)
    C = np.cos(np.pi * np.outer(i, i) / (NX - 1))
    lam = 2.0 * np.cos(np.pi * i / (NX - 1))
    w = np.ones(NX)
    w[0] = w[-1] = 0.5
    s = math.sqrt(2.0 / (NX - 1))
    # C^{-1} = (2/(N-1)) W C W; balance fp16 range: A1 = Cinv/s, B1 = C*s
    A1 = (2.0 / (NX - 1) / s) * (w[:, None] * C * w[None, :])
    B1 = C * s
    R = K // 2
    kept = np.r_[0:R, NX - R:NX]
    kperm = np.r_[kept[kept % 2 == 0], kept[kept % 2 == 1]]  # evens, then odds
    A1t = A1[kperm, :512].T               # [512 (i'/j'), K]   fwd weights
    B1t = B1[:512, :][:, kperm].T         # [K, 512 (j'/i')]   inv weights
    Lam = ((lam[kperm][:, None] + lam[kperm][None, :]) / 4.0) ** n_iter
    KB = K // 128
    # WA[c, cblk*K + k]   = A1t[cblk*128 + c, k]      (cblk: spatial block)
    # WB[c, kblk*512 + f] = B1t[kblk*128 + c, f]      (kblk: mode block)
    WA = A1t.reshape(4, 128, K).transpose(1, 0, 2)
    WA = np.ascontiguousarray(WA.reshape(128, 4 * K)).astype(np.float16)
    WB = B1t.reshape(KB, 128, 512).transpose(1, 0, 2)
    WB = np.ascontiguousarray(WB.reshape(128, KB * 512)).astype(np.float16)
    LAM = Lam.reshape(KB, 128, K).transpose(1, 0, 2)
    LAM = np.ascontiguousarray(LAM.reshape(128, KB * K)).astype(np.float32)
    return {"wa": WA, "wb": WB, "lam": LAM}


def _build_program(n_iter):
    import concourse.bacc as bacc
    import concourse.mybir as mybir
    import concourse.tile as tile

    K = _choose_K(n_iter)
    KB = K // 128          # mode blocks (parity-permuted: KB/2 even, KB/2 odd)
    KH = KB // 2           # blocks per parity
    KP = K // 2            # modes per parity
    nslots = 2 if K <= 512 else 1
    f16 = mybir.dt.float16
    f32 = mybir.dt.float32
    mult = mybir.AluOpType.mult
    add = mybir.AluOpType.add
    sub = mybir.AluOpType.subtract

    nc = bacc.Bacc("TRN2", target_bir_lowering=False, debug=False)
    # x0: per image the exact SBUF layout [128, 16*512] (quadrant q, block ci
    # at cols (q*4+ci)*512); shape-preserving DMAs only
    x0_d = nc.dram_tensor("x0", [IMGS_PER_CORE * 128, 16 * 512], f16,
                          kind="ExternalInput").ap()
    wa_d = nc.dram_tensor("wa", [128, 4 * K], f16, kind="ExternalInput").ap()
    wb_d = nc.dram_tensor("wb", [128, KB * 512], f16,
                          kind="ExternalInput").ap()
    lam_d = nc.dram_tensor("lam", [128, KB * K], f32, kind="ExternalInput").ap()
    # y: raw folded output [1024, 1024] per image (host unfolds)
    y_d = nc.dram_tensor("y", [IMGS_PER_CORE * NX, NX], f16,
                         kind="ExternalOutput").ap()

    with tile.TileContext(nc) as tc, ExitStack() as ctx:
        wp = ctx.enter_context(tc.tile_pool(name="w", bufs=1))
        bp = ctx.enter_context(tc.tile_pool(name="b", bufs=1))
        psum_bufs = 8 if K <= 512 else 4
        pmm = ctx.enter_context(tc.tile_pool(name="pmm", bufs=psum_bufs,
                                             space="PSUM"))
        sp = ctx.enter_context(tc.tile_pool(name="sp", bufs=6))

        WA = wp.tile([128, 4 * K], f16)
        WB = wp.tile([128, KB * 512], f16)
        LAM = wp.tile([128, KB * K], f32)

        # Xq: 16 blocks of [128, 512]: quadrant q (a=i-parity, b=j-parity,
        # q = 2a + b), block = q*4 + ci
        Xq = [bp.tile([128, 16 * 512], f16, name=f"x{s}") for s in range(nslots)]
        # UT_p[j', kv] (vertical modes already transposed): col = jb*K + kv
        UTe = [bp.tile([128, 4 * K], f16, name=f"ute{s}") for s in range(nslots)]
        UTo = [bp.tile([128, 4 * K], f16, name=f"uto{s}") for s in range(nslots)]
        # WC[kh, kv] scaled by Lam: col = khblk*K + kv
        WC = [bp.tile([128, KB * K], f16, name=f"wc{s}") for s in range(nslots)]
        # ZT[kv, col]: col<512 = j' (sym part), col>=512 = j' (antisym part)
        ZT = [bp.tile([128, KB * NX], f16, name=f"zt{s}") for s in range(nslots)]
        Ol = [bp.tile([128, 4 * NX], f16, name=f"ol{s}") for s in range(nslots)]
        Oh = [bp.tile([128, 4 * NX], f16, name=f"oh{s}") for s in range(nslots)]

        # PE warmup: ramp the tensor engine's pstate on zeros while the
        # first input/weight DMAs are still in flight
        Wz = bp.tile([128, 512], f16, name="wz")
        nc.gpsimd.memset(Wz[:], 0.0)
        pw = pmm.tile([128, 512], f32, name="pw", tag="mm")
        for r in range(8):
            nc.tensor.matmul(pw[:], Wz[:, :128], Wz[:],
                             start=(r == 0), stop=(r == 7))

        def load_x(s, img):
            r0 = img * 128
            for h in range(2):           # one DMA per pj half [128, 4096]
                nc.sync.dma_start(Xq[s][:, h * 4096:(h + 1) * 4096],
                                  x0_d[r0:r0 + 128, h * 4096:(h + 1) * 4096])

        def passA(s):
            # UT_pj[j', k] = sum_{i'} Xq[par(k), pj][i', j'] * A1t[i', k]
            # lhsT = input quadrant block, rhs = WA slice; even/odd k halves
            # accumulate into the two col-halves of one PSUM bank
            for pj in range(2):
                for jb in range(4):
                    p = pmm.tile([128, 2 * KP], f32, name="pm", tag="mm")
                    for par in range(2):           # k parity: even, odd
                        pos = 2 * pj + par
                        for ci in range(4):
                            nc.tensor.matmul(
                                p[:, par * KP:(par + 1) * KP],
                                Xq[s][:, (pos * 4 + ci) * 512 + jb * 128:
                                       (pos * 4 + ci) * 512 + jb * 128 + 128],
                                WA[:, ci * K + par * KP: ci * K + (par + 1) * KP],
                                start=(ci == 0), stop=(ci == 3))
                    dst = UTe[s] if pj == 0 else UTo[s]
                    nc.scalar.copy(dst[:, jb * K:(jb + 1) * K], p[:])

        def passC(s):
            # WC[kh, kv] = Lam * sum_{j'} A1p[kh, j'] UT_{par(kh)}[j', kv]
            for ko in range(KB):
                rhs = UTe[s] if ko < KH else UTo[s]
                for f0 in range(0, K, 512):
                    fw = min(512, K - f0)
                    p = pmm.tile([128, fw], f32, name="pm", tag="mm")
                    for jb in range(4):
                        nc.tensor.matmul(
                            p[:], WA[:, jb * K + ko * 128: jb * K + ko * 128 + 128],
                            rhs[:, jb * K + f0: jb * K + f0 + fw],
                            start=(jb == 0), stop=(jb == 3))
                    nc.vector.tensor_tensor(
                        WC[s][:, ko * K + f0: ko * K + f0 + fw], p[:],
                        LAM[:, ko * K + f0: ko * K + f0 + fw], op=mult)

        def passE(s):
            # ZeT/ZoT[kv, j'] = sum_{kh even/odd} WC[kh, kv] B1t[kh, j']
            # lhsT = WC block, rhs = WB slice.
            # ZT[:, kb*NX + :512] = ZeT + ZoT (Z at j'), + 512: = ZeT - ZoT
            for kvb in range(KB):
                pe = pmm.tile([128, 512], f32, name="pe", tag="mm")
                po = pmm.tile([128, 512], f32, name="po", tag="mm")
                for kb in range(KH):
                    nc.tensor.matmul(
                        pe[:], WC[s][:, kb * K + kvb * 128: kb * K + kvb * 128 + 128],
                        WB[:, kb * 512:(kb + 1) * 512],
                        start=(kb == 0), stop=(kb == KH - 1))
                for kb in range(KH, KB):
                    nc.tensor.matmul(
                        po[:], WC[s][:, kb * K + kvb * 128: kb * K + kvb * 128 + 128],
                        WB[:, kb * 512:(kb + 1) * 512],
                        start=(kb == KH), stop=(kb == KB - 1))
                ps = sp.tile([128, 512], f16, name="ps", tag="ps")
                nc.scalar.mul(ps[:], po[:], 2.0)
                nc.vector.scalar_tensor_tensor(
                    ZT[s][:, kvb * NX: kvb * NX + 512], ps[:], 0.5, pe[:],
                    op0=mult, op1=add)
                nc.gpsimd.tensor_tensor(
                    ZT[s][:, kvb * NX + 512: (kvb + 1) * NX],
                    ZT[s][:, kvb * NX: kvb * NX + 512], ps[:], op=sub)

        def passG(s, img):
            # Oe/Oo[i', col] = sum_{kv even/odd} B1p[i', kv] ZT[kv, col]
            # Ol = Oe + Oo (= out[i']), Oh = Oe - Oo (= out[1023-i'])
            r0 = img * NX
            for io in range(4):
                for hf in range(2):
                    pe = pmm.tile([128, 512], f32, name="pe", tag="mm")
                    po = pmm.tile([128, 512], f32, name="po", tag="mm")
                    for kb in range(KH):
                        o = kb * 512 + io * 128
                        nc.tensor.matmul(
                            pe[:], WB[:, o:o + 128],
                            ZT[s][:, kb * NX + hf * 512: kb * NX + hf * 512 + 512],
                            start=(kb == 0), stop=(kb == KH - 1))
                    for kb in range(KH, KB):
                        o = kb * 512 + io * 128
                        nc.tensor.matmul(
                            po[:], WB[:, o:o + 128],
                            ZT[s][:, kb * NX + hf * 512: kb * NX + hf * 512 + 512],
                            start=(kb == KH), stop=(kb == KB - 1))
                    c0 = io * NX + hf * 512
                    ps = sp.tile([128, 512], f16, name="ps", tag="ps")
                    nc.scalar.mul(ps[:], po[:], 2.0)
                    nc.vector.scalar_tensor_tensor(
                        Ol[s][:, c0:c0 + 512], ps[:], 0.5, pe[:],
                        op0=mult, op1=add)
                    nc.gpsimd.tensor_tensor(
                        Oh[s][:, c0:c0 + 512], Ol[s][:, c0:c0 + 512], ps[:],
                        op=sub)
                nc.sync.dma_start(y_d[r0 + io * 128: r0 + (io + 1) * 128, :],
                                  Ol[s][:, io * NX:(io + 1) * NX])
                nc.sync.dma_start(
                    y_d[r0 + 512 + io * 128: r0 + 512 + (io + 1) * 128, :],
                    Oh[s][:, io * NX:(io + 1) * NX])

        # weights on the ACT HWDGE queue so issue/transfer overlaps X loads
        nc.scalar.dma_start(WA[:], wa_d[:, :])
        load_x(0, 0)
        nc.scalar.dma_start(WB[:], wb_d[:, :])
        nc.scalar.dma_start(LAM[:], lam_d[:, :])
        if nslots == 2:
            load_x(1, 1)
            passA(0); passA(1)
            passC(0); passC(1)
            passE(0); passE(1)
            passG(0, 0); passG(1, 1)
        else:
            for img in range(IMGS_PER_CORE):
                if img:
                    load_x(0, img)
                passA(0); passC(0); passE(0); passG(0, img)

    nc.compile()
    return nc, _host_weights(n_iter, _choose_K(n_iter))


def _fold_input(x_f32):
    """[16, NX, NX] f32 -> [16, 128, 8192] f16 parity quadrants in the
    device SBUF layout: col = (q*4 + ci)*512 + j', partition = i' % 128."""
    lo = x_f32[:, :512, :]
    hi = x_f32[:, 1023:511:-1, :]
    ia = lo + hi    # i-even
    ib = lo - hi    # i-odd
    quad = np.empty((16, 4, 512, 512), np.float32)
    for q, part in ((0, ia), (2, ib)):
        quad[:, q] = part[:, :, :512] + part[:, :, 1023:511:-1]
        quad[:, q + 1] = part[:, :, :512] - part[:, :, 1023:511:-1]
    quad = quad[:, [0, 2, 1, 3]]     # pj-major device order
    # [16, qpos, ci*128+p, j'] -> [16, p, qpos, ci, j']
    quad = quad.reshape(16, 4, 4, 128, 512).transpose(0, 3, 1, 2, 4)
    return np.ascontiguousarray(quad.reshape(16, 128, 8192)).astype(np.float16)


_PERM = np.r_[0:512, 1023:511:-1]


def _make_in_maps(x_f32, n_iter):
    """x_f32: [16, NX, NX] float32. Returns (nc, in_maps)."""
    if n_iter not in _compiled_cache:
        _compiled_cache[n_iter] = _build_program(n_iter)
    nc, wdict = _compiled_cache[n_iter]
    xq = _fold_input(x_f32)
    in_maps = []
    for c in range(N_CORES):
        shard = np.ascontiguousarray(
            xq[c * IMGS_PER_CORE:(c + 1) * IMGS_PER_CORE].reshape(
                IMGS_PER_CORE * 128, 16 * 512))
        m = {"x0": shard}
        m.update(wdict)
        in_maps.append(m)
    return nc, in_maps


def kernel(layout, heat, n_iter):
    n_iter = int(n_iter)
    heat = np.asarray(heat)
    out_shape = heat.shape
    x = np.asarray(heat, np.float32).reshape(16, NX, NX)
    if n_iter <= 0:
        return x.reshape(out_shape).copy()

    from concourse.bass_utils import run_bass_kernel_spmd

    nc, in_maps = _make_in_maps(x, n_iter)
    res = run_bass_kernel_spmd(nc, in_maps, core_ids=list(range(N_CORES)))
    out = np.empty((16, NX, NX), np.float32)
    for c in range(N_CORES):
        raw = res.results[c]["y"].astype(np.float32).reshape(
            IMGS_PER_CORE, NX, NX)
        # unfold: raw row/col r>=512 holds index 1535-r
        out[c * IMGS_PER_CORE:(c + 1) * IMGS_PER_CORE] = (
            raw[:, _PERM][:, :, _PERM])
    return out.reshape(out_shape)


# revision 30
# speedup vs baseline: 1.1212x; 1.0282x over previous
"""Trainium2 Bass kernel for n-iteration Jacobi (3x3 cross stencil, reflect pad).

x_{t+1} = 0.25*(V + H) x_t + f,  f = COF*layout (|f| ~ 2.4e-9, contributes
< 3e-6 relative to the output; dropped).

V (vertical) and H (horizontal) neighbor-sum operators with this reflect
boundary are exactly diagonalized by the DCT-I basis v_k[i] = cos(pi*i*k/1023),
eigenvalues lam_k = 2*cos(pi*k/1023).  n Jacobi iterations collapse to one
spectral sandwich per image:

    out = C_k @ (Lam2D * (Cinv_k @ X @ Cinv_k^T)) @ C_k^T
    Lam2D[a,b] = ((lam_a + lam_b)/4)^n

Three reductions on top of the plain sandwich:
  1. Mode truncation: Lam2D^n decays doubly-exponentially away from the
     lowest/highest frequencies; keep K=512 of 1024 modes per axis for n=50
     (max truncated |Lam| ~ 4e-4).
  2. Even/odd folding: cos(pi*k*(1023-i)/1023) = (-1)^k cos(pi*k*i/1023),
     so folding the spatial axes into symmetric/antisymmetric halves halves
     every contraction.  The input fold is done on the host (images are sent
     as 4 parity quadrants), the intermediate parity recombines fuse into the
     PSUM copy-outs as add/sub pairs, and the output unfold is a host-side
     index permutation.
  3. No PE transposes: the two passes that would need transposed outputs
     (forward-vertical, inverse-vertical) run with the *data* as the
     stationary lhsT operand and the transform matrix as the moving rhs,
     which yields the transposed orientation directly.

Per image: 4 half-contraction matmul passes, ~49K PE rows at 1 row/cycle
(vs ~1.25M rows for iterated banded-matmul stepping).  All matmul operands
fp16 (PSUM accumulates fp32); measured error vs the fp64 reference ~7e-4
max-rel.  Per core: 2 of 16 images, passes software-pipelined across the
two images.
"""

import math
from contextlib import ExitStack

import numpy as np

NX = 1024
N_CORES = 8
IMGS_PER_CORE = 2
LN_TAU = math.log(1e4)

_compiled_cache = {}


def _choose_K(n_iter):
    # keep modes with ((lam_a+lam_b)/4)^n >= 1e-4; parity folding needs
    # K to be a multiple of 256
    R = int(math.ceil(1023.0 / math.pi * math.sqrt(2.0 * LN_TAU / max(n_iter, 1))))
    K = min(1024, ((2 * R + 255) // 256) * 256)
    return K


def _host_weights(n_iter, K):
    i = np.arange(NX)
    C = np.cos(np.pi * np.outer(i, i) / (NX - 1))
    lam = 2.0 * np.cos(np.pi * i / (NX - 1))
    w = np.ones(NX)
    w[0] = w[-1] = 0.5
    s = math.sqrt(2.0 / (NX - 1))
    # C^{-1} = (2/(N-1)) W C W; balance fp16 range: A1 = Cinv/s, B1 = C*s
    A1 = (2.0 / (NX - 1) / s) * (w[:, None] * C * w[None, :])
    B1 = C * s
    R = K // 2
    kept = np.r_[0:R, NX - R:NX]
    kperm = np.r_[kept[kept % 2 == 0], kept[kept % 2 == 1]]  # evens, then odds
    A1t = A1[kperm, :512].T               # [512 (i'/j'), K]   fwd weights
    B1t = B1[:512, :][:, kperm].T         # [K, 512 (j'/i')]   inv weights
    Lam = ((lam[kperm][:, None] + lam[kperm][None, :]) / 4.0) ** n_iter
    KB = K // 128
    # WA[c, cblk*K + k]   = A1t[cblk*128 + c, k]      (cblk: spatial block)
    # WB[c, kblk*512 + f] = B1t[kblk*128 + c, f]      (kblk: mode block)
    WA = A1t.reshape(4, 128, K).transpose(1, 0, 2)
    WA = np.ascontiguousarray(WA.reshape(128, 4 * K)).astype(np.float16)
    WB = B1t.reshape(KB, 128, 512).transpose(1, 0, 2)
    WB = np.ascontiguousarray(WB.reshape(128, KB * 512)).astype(np.float16)
    LAM = Lam.reshape(KB, 128, K).transpose(1, 0, 2)
    LAM = np.ascontiguousarray(LAM.reshape(128, KB * K)).astype(np.float32)
    return {"wa": WA, "wb": WB, "lam": LAM}


def _build_program(n_iter):
    import concourse.bacc as bacc
    import concourse.mybir as mybir
    import concourse.tile as tile

    K = _choose_K(n_iter)
    KB = K // 128          # mode blocks (parity-permuted: KB/2 even, KB/2 odd)
    KH = KB // 2           # blocks per parity
    KP = K // 2            # modes per parity
    nslots = 2 if K <= 512 else 1
    f16 = mybir.dt.float16
    f32 = mybir.dt.float32
    mult = mybir.AluOpType.mult
    add = mybir.AluOpType.add
    sub = mybir.AluOpType.subtract

    nc = bacc.Bacc("TRN2", target_bir_lowering=False, debug=False)
    # x0: per image the exact SBUF layout [128, 16*512] (quadrant q, block ci
    # at cols (q*4+ci)*512); shape-preserving DMAs only
    x0_d = nc.dram_tensor("x0", [IMGS_PER_CORE * 128, 16 * 512], f16,
                          kind="ExternalInput").ap()
    wa_d = nc.dram_tensor("wa", [128, 4 * K], f16, kind="ExternalInput").ap()
    wb_d = nc.dram_tensor("wb", [128, KB * 512], f16,
                          kind="ExternalInput").ap()
    lam_d = nc.dram_tensor("lam", [128, KB * K], f32, kind="ExternalInput").ap()
    # y: raw folded output [1024, 1024] per image (host unfolds)
    y_d = nc.dram_tensor("y", [IMGS_PER_CORE * NX, NX], f16,
                         kind="ExternalOutput").ap()

    with tile.TileContext(nc) as tc, ExitStack() as ctx:
        wp = ctx.enter_context(tc.tile_pool(name="w", bufs=1))
        bp = ctx.enter_context(tc.tile_pool(name="b", bufs=1))
        psum_bufs = 8 if K <= 512 else 4
        pmm = ctx.enter_context(tc.tile_pool(name="pmm", bufs=psum_bufs,
                                             space="PSUM"))
        sp = ctx.enter_context(tc.tile_pool(name="sp", bufs=6))

        WA = wp.tile([128, 4 * K], f16)
        WB = wp.tile([128, KB * 512], f16)
        LAM = wp.tile([128, KB * K], f32)

        # Xq: 16 blocks of [128, 512]: quadrant q (a=i-parity, b=j-parity,
        # q = 2a + b), block = q*4 + ci
        Xq = [bp.tile([128, 16 * 512], f16, name=f"x{s}") for s in range(nslots)]
        # UT_p[j', kv] (vertical modes already transposed): col = jb*K + kv
        UTe = [bp.tile([128, 4 * K], f16, name=f"ute{s}") for s in range(nslots)]
        UTo = [bp.tile([128, 4 * K], f16, name=f"uto{s}") for s in range(nslots)]
        # WC[kh, kv] scaled by Lam: col = khblk*K + kv
        WC = [bp.tile([128, KB * K], f16, name=f"wc{s}") for s in range(nslots)]
        # ZT[kv, col]: col<512 = j' (sym part), col>=512 = j' (antisym part)
        ZT = [bp.tile([128, KB * NX], f16, name=f"zt{s}") for s in range(nslots)]
        Ol = [bp.tile([128, 4 * NX], f16, name=f"ol{s}") for s in range(nslots)]
        Oh = [bp.tile([128, 4 * NX], f16, name=f"oh{s}") for s in range(nslots)]

        # PE warmup: ramp the tensor engine's pstate on zeros while the
        # first input/weight DMAs are still in flight
        Wz = bp.tile([128, 512], f16, name="wz")
        nc.gpsimd.memset(Wz[:], 0.0)
        pw = pmm.tile([128, 512], f32, name="pw", tag="mm")
        for r in range(8):
            nc.tensor.matmul(pw[:], Wz[:, :128], Wz[:],
                             start=(r == 0), stop=(r == 7))
        # fine-grained filler so the queue can drain the moment data lands
        pw2 = pmm.tile([128, 512], f32, name="pw2", tag="mm")
        for r in range(24):
            nc.tensor.matmul(pw2[:, :128], Wz[:, :128], Wz[:, :128],
                             start=(r == 0), stop=(r == 23))

        def load_x(s, img):
            r0 = img * 128
            for h in range(2):           # one DMA per pj half [128, 4096]
                nc.sync.dma_start(Xq[s][:, h * 4096:(h + 1) * 4096],
                                  x0_d[r0:r0 + 128, h * 4096:(h + 1) * 4096])

        def passA(s):
            # UT_pj[j', k] = sum_{i'} Xq[par(k), pj][i', j'] * A1t[i', k]
            # lhsT = input quadrant block, rhs = WA slice; even/odd k halves
            # accumulate into the two col-halves of one PSUM bank
            for pj in range(2):
                for jb in range(4):
                    p = pmm.tile([128, 2 * KP], f32, name="pm", tag="mm")
                    for par in range(2):           # k parity: even, odd
                        pos = 2 * pj + par
                        for ci in range(4):
                            nc.tensor.matmul(
                                p[:, par * KP:(par + 1) * KP],
                                Xq[s][:, (pos * 4 + ci) * 512 + jb * 128:
                                       (pos * 4 + ci) * 512 + jb * 128 + 128],
                                WA[:, ci * K + par * KP: ci * K + (par + 1) * KP],
                                start=(ci == 0), stop=(ci == 3))
                    dst = UTe[s] if pj == 0 else UTo[s]
                    nc.scalar.copy(dst[:, jb * K:(jb + 1) * K], p[:])

        def passC(s):
            # WC[kh, kv] = Lam * sum_{j'} A1p[kh, j'] UT_{par(kh)}[j', kv]
            for ko in range(KB):
                rhs = UTe[s] if ko < KH else UTo[s]
                for f0 in range(0, K, 512):
                    fw = min(512, K - f0)
                    p = pmm.tile([128, fw], f32, name="pm", tag="mm")
                    for jb in range(4):
                        nc.tensor.matmul(
                            p[:], WA[:, jb * K + ko * 128: jb * K + ko * 128 + 128],
                            rhs[:, jb * K + f0: jb * K + f0 + fw],
                            start=(jb == 0), stop=(jb == 3))
                    nc.vector.tensor_tensor(
                        WC[s][:, ko * K + f0: ko * K + f0 + fw], p[:],
                        LAM[:, ko * K + f0: ko * K + f0 + fw], op=mult)

        def passE(s):
            # ZeT/ZoT[kv, j'] = sum_{kh even/odd} WC[kh, kv] B1t[kh, j']
            # lhsT = WC block, rhs = WB slice.
            # ZT[:, kb*NX + :512] = ZeT + ZoT (Z at j'), + 512: = ZeT - ZoT
            for kvb in range(KB):
                pe = pmm.tile([128, 512], f32, name="pe", tag="mm")
                po = pmm.tile([128, 512], f32, name="po", tag="mm")
                for kb in range(KH):
                    nc.tensor.matmul(
                        pe[:], WC[s][:, kb * K + kvb * 128: kb * K + kvb * 128 + 128],
                        WB[:, kb * 512:(kb + 1) * 512],
                        start=(kb == 0), stop=(kb == KH - 1))
                for kb in range(KH, KB):
                    nc.tensor.matmul(
                        po[:], WC[s][:, kb * K + kvb * 128: kb * K + kvb * 128 + 128],
                        WB[:, kb * 512:(kb + 1) * 512],
                        start=(kb == KH), stop=(kb == KB - 1))
                ps = sp.tile([128, 512], f16, name="ps", tag="ps")
                nc.scalar.mul(ps[:], po[:], 2.0)
                nc.vector.scalar_tensor_tensor(
                    ZT[s][:, kvb * NX: kvb * NX + 512], ps[:], 0.5, pe[:],
                    op0=mult, op1=add)
                nc.gpsimd.tensor_tensor(
                    ZT[s][:, kvb * NX + 512: (kvb + 1) * NX],
                    ZT[s][:, kvb * NX: kvb * NX + 512], ps[:], op=sub)

        def passG(s, img):
            # Oe/Oo[i', col] = sum_{kv even/odd} B1p[i', kv] ZT[kv, col]
            # Ol = Oe + Oo (= out[i']), Oh = Oe - Oo (= out[1023-i'])
            r0 = img * NX
            for io in range(4):
                for hf in range(2):
                    pe = pmm.tile([128, 512], f32, name="pe", tag="mm")
                    po = pmm.tile([128, 512], f32, name="po", tag="mm")
                    for kb in range(KH):
                        o = kb * 512 + io * 128
                        nc.tensor.matmul(
                            pe[:], WB[:, o:o + 128],
                            ZT[s][:, kb * NX + hf * 512: kb * NX + hf * 512 + 512],
                            start=(kb == 0), stop=(kb == KH - 1))
                    for kb in range(KH, KB):
                        o = kb * 512 + io * 128
                        nc.tensor.matmul(
                            po[:], WB[:, o:o + 128],
                            ZT[s][:, kb * NX + hf * 512: kb * NX + hf * 512 + 512],
                            start=(kb == KH), stop=(kb == KB - 1))
                    c0 = io * NX + hf * 512
                    ps = sp.tile([128, 512], f16, name="ps", tag="ps")
                    nc.scalar.mul(ps[:], po[:], 2.0)
                    nc.vector.scalar_tensor_tensor(
                        Ol[s][:, c0:c0 + 512], ps[:], 0.5, pe[:],
                        op0=mult, op1=add)
                    if s == nslots - 1 and io >= 2:
                        nc.vector.tensor_tensor(
                            Oh[s][:, c0:c0 + 512], Ol[s][:, c0:c0 + 512],
                            ps[:], op=sub)
                    else:
                        nc.gpsimd.tensor_tensor(
                            Oh[s][:, c0:c0 + 512], Ol[s][:, c0:c0 + 512],
                            ps[:], op=sub)
                nc.sync.dma_start(y_d[r0 + io * 128: r0 + (io + 1) * 128, :],
                                  Ol[s][:, io * NX:(io + 1) * NX])
                nc.sync.dma_start(
                    y_d[r0 + 512 + io * 128: r0 + 512 + (io + 1) * 128, :],
                    Oh[s][:, io * NX:(io + 1) * NX])

        # weights on the ACT HWDGE queue so issue/transfer overlaps X loads
        nc.scalar.dma_start(WA[:], wa_d[:, :])
        load_x(0, 0)
        nc.scalar.dma_start(WB[:], wb_d[:, :])
        nc.scalar.dma_start(LAM[:], lam_d[:, :])
        if nslots == 2:
            load_x(1, 1)
            passA(0); passA(1)
            passC(0); passC(1)
            passE(0); passE(1)
            passG(0, 0); passG(1, 1)
        else:
            for img in range(IMGS_PER_CORE):
                if img:
                    load_x(0, img)
                passA(0); passC(0); passE(0); passG(0, img)

    nc.compile()
    return nc, _host_weights(n_iter, _choose_K(n_iter))


def _fold_input(x_f32):
    """[16, NX, NX] f32 -> [16, 128, 8192] f16 parity quadrants in the
    device SBUF layout: col = (q*4 + ci)*512 + j', partition = i' % 128."""
    lo = x_f32[:, :512, :]
    hi = x_f32[:, 1023:511:-1, :]
    ia = lo + hi    # i-even
    ib = lo - hi    # i-odd
    quad = np.empty((16, 4, 512, 512), np.float32)
    for q, part in ((0, ia), (2, ib)):
        quad[:, q] = part[:, :, :512] + part[:, :, 1023:511:-1]
        quad[:, q + 1] = part[:, :, :512] - part[:, :, 1023:511:-1]
    quad = quad[:, [0, 2, 1, 3]]     # pj-major device order
    # [16, qpos, ci*128+p, j'] -> [16, p, qpos, ci, j']
    quad = quad.reshape(16, 4, 4, 128, 512).transpose(0, 3, 1, 2, 4)
    return np.ascontiguousarray(quad.reshape(16, 128, 8192)).astype(np.float16)


_PERM = np.r_[0:512, 1023:511:-1]


def _make_in_maps(x_f32, n_iter):
    """x_f32: [16, NX, NX] float32. Returns (nc, in_maps)."""
    if n_iter not in _compiled_cache:
        _compiled_cache[n_iter] = _build_program(n_iter)
    nc, wdict = _compiled_cache[n_iter]
    xq = _fold_input(x_f32)
    in_maps = []
    for c in range(N_CORES):
        shard = np.ascontiguousarray(
            xq[c * IMGS_PER_CORE:(c + 1) * IMGS_PER_CORE].reshape(
                IMGS_PER_CORE * 128, 16 * 512))
        m = {"x0": shard}
        m.update(wdict)
        in_maps.append(m)
    return nc, in_maps


def kernel(layout, heat, n_iter):
    n_iter = int(n_iter)
    heat = np.asarray(heat)
    out_shape = heat.shape
    x = np.asarray(heat, np.float32).reshape(16, NX, NX)
    if n_iter <= 0:
        return x.reshape(out_shape).copy()

    from concourse.bass_utils import run_bass_kernel_spmd

    nc, in_maps = _make_in_maps(x, n_iter)
    res = run_bass_kernel_spmd(nc, in_maps, core_ids=list(range(N_CORES)))
    out = np.empty((16, NX, NX), np.float32)
    for c in range(N_CORES):
        raw = res.results[c]["y"].astype(np.float32).reshape(
            IMGS_PER_CORE, NX, NX)
        # unfold: raw row/col r>=512 holds index 1535-r
        out[c * IMGS_PER_CORE:(c + 1) * IMGS_PER_CORE] = (
            raw[:, _PERM][:, :, _PERM])
    return out.reshape(out_shape)
